# revision 1
# baseline (speedup 1.0000x reference)
"""Trainium2 Bass kernel for nn_DepthDCOp (per-pixel depthwise dynamic conv).

out[n,c,h,w] = sum_{i,j in 0..2} kernel[n,0,i*3+j,h,w] * xpad[n,c,h+i,w+j]
  (3x3 stencil, zero padding, per-pixel weights shared across channels)

Sharding: data-parallel over N — core i computes sample i (N == 8 == n_cores).

Per-core design (bf16 in/out, fp32 PSUM accumulate):
  The stencil is recast as banded matmuls over the flattened hw axis.  For
  output pixels g = 128a+p (tile a), out^T[g, c] = sum_t k_t[g] *
  x^T[g + d_t, c] with tap offsets d_t in {-65..65}.  The host packs the
  per-pixel weights into band matrices B[a, b][q, p] = k_t[128a+p] at
  q = p + d_t - 128(b-1) (w-edge taps zeroed, h-edges fall outside the
  band), so each output tile is just

      out^T_a = L_a^T @ x^T_{a-1} + C_a^T @ x^T_a + R_a^T @ x^T_{a+1}

  i.e. three 128-contraction matmuls accumulating in PSUM.  The L/R halo
  matrices only have 65 nonzero contraction rows, so only those rows are
  DMA'd (the pad rows are memset once by the idle Pool engine —
  contraction size costs no PE time).  The PE does the shift+multiply+
  9-tap-reduce in one pass; ACT/DVE alternate on the PSUM->SBUF drains;
  the gapless DMA stream (x^T in, bands in, out^T out, all bf16; 6.1 MB
  at the 360 GB/s model roofline) is the bottleneck end to end, matching
  the memory target regime.  x/out transposes happen on the host.
"""

import os
import sys

import numpy as np
import ml_dtypes

for _p in ("/opt/trn_rl_repo", "/root/.axon_site/_ro/trn_rl_repo"):
    if os.path.isdir(_p) and _p not in sys.path:
        sys.path.insert(0, _p)

import concourse.bass as bass  # noqa: E402
import concourse.bacc as bacc  # noqa: E402
import concourse.mybir as mybir  # noqa: E402
import concourse.tile as tile  # noqa: E402
from concourse.bass_utils import run_bass_kernel_spmd  # noqa: E402

N, C, H, W = 8, 256, 64, 64
HW = H * W  # 4096
K = 3
T = K * K  # 9 taps
BF16 = mybir.dt.bfloat16
F32 = mybir.dt.float32

P = 128           # pixels per tile (partition dim of out^T tiles)
HR = 65           # real (nonzero) contraction rows per L/R halo block
NT = HW // P      # 32 hw tiles
XC = 8            # x tiles per input DMA chunk
NCX = NT // XC    # 4 x chunks
OC = 4            # out tiles per output DMA chunk
NCO = NT // OC    # 8 out chunks
BC = 8            # band tiles per DMA chunk
NCB = NT // BC    # 4 band chunks

_cached = {}


def _build_nc():
    nc = bacc.Bacc(trn_type="TRN2")
    xT_d = nc.dram_tensor("xT", [HW, C], BF16, kind="ExternalInput")
    c_d = nc.dram_tensor("bandC", [P, NT * P], BF16, kind="ExternalInput")
    l_d = nc.dram_tensor("bandL", [HR, NT * P], BF16, kind="ExternalInput")
    r_d = nc.dram_tensor("bandR", [HR, NT * P], BF16, kind="ExternalInput")
    oT_d = nc.dram_tensor("outT", [HW, C], BF16, kind="ExternalOutput")

    with tile.TileContext(nc) as tc:
        with (
            tc.tile_pool(name="xp", bufs=1) as xp,
            tc.tile_pool(name="bp", bufs=1) as bp,
            tc.tile_pool(name="op", bufs=8) as op,
            tc.tile_pool(name="pso", bufs=8, space="PSUM") as pso,
        ):
            # Per-chunk SBUF tiles (separate tiles => DMA/compute overlap at
            # chunk granularity in the tile dependency tracker).
            xts = [
                xp.tile([P, XC, C], BF16, name=f"xt{s}") for s in range(NCX)
            ]
            cts = [
                bp.tile([P, BC, P], BF16, name=f"ct{s}") for s in range(NCB)
            ]
            # The L/R halo stationaries are full 128-row matrices whose 63
            # pad rows are zeroed once by Pool memsets (the engine partition
            # addressing rules bar partial-partition matmul reads off base
            # 0, and contraction size doesn't cost PE time) — the band DMAs
            # only ship the 65 real rows.
            lts = [
                bp.tile([P, BC, P], BF16, name=f"lt{s}") for s in range(NCB)
            ]
            rts = [
                bp.tile([P, BC, P], BF16, name=f"rt{s}") for s in range(NCB)
            ]
            # Each memset covers one extra row that the band DMA then
            # overwrites (64-partition aligned ranges).
            for s in range(NCB):
                nc.gpsimd.memset(lts[s][0 : P - HR + 1, :, :], 0.0)
                nc.gpsimd.memset(rts[s][HR - 1 : P, :, :], 0.0)
            xr = xT_d.rearrange("(a p) c -> p a c", p=P)
            cr = c_d.rearrange("q (a p) -> q a p", p=P)
            lr = l_d.rearrange("q (a p) -> q a p", p=P)
            rr = r_d.rearrange("q (a p) -> q a p", p=P)
            # Interleave so early tiles' inputs land first while keeping the
            # stream gapless.
            for s in range(NCB):
                nc.sync.dma_start(
                    xts[s][:, :, :], xr[:, s * XC : (s + 1) * XC, :]
                )
                nc.sync.dma_start(cts[s][:, :, :], cr[:, s * BC : (s + 1) * BC, :])
                nc.sync.dma_start(
                    lts[s][P - HR : P, :, :], lr[:, s * BC : (s + 1) * BC, :]
                )
                nc.sync.dma_start(
                    rts[s][0:HR, :, :], rr[:, s * BC : (s + 1) * BC, :]
                )

            orr = oT_d.rearrange("(a p) c -> p a c", p=P)
            for s in range(NCO):
                ot = op.tile([P, OC, C], BF16, tag="ot", name=f"ot{s}")
                for i in range(OC):
                    a = s * OC + i
                    po = pso.tile([P, C], F32, tag="po", name=f"po{a}")
                    sb, ib = a // BC, a % BC
                    first, last = (a == 0), (a == NT - 1)
                    if not first:
                        m = a - 1
                        nc.tensor.matmul(
                            po[:, :],
                            lts[sb][:, ib, :],
                            xts[m // XC][:, m % XC, :],
                            start=True,
                            stop=False,
                        )
                    nc.tensor.matmul(
                        po[:, :],
                        cts[sb][:, ib, :],
                        xts[a // XC][:, a % XC, :],
                        start=first,
                        stop=last,
                    )
                    if not last:
                        m = a + 1
                        nc.tensor.matmul(
                            po[:, :],
                            rts[sb][:, ib, :],
                            xts[m // XC][:, m % XC, :],
                            start=False,
                            stop=True,
                        )
                    # Alternate drain engine so neither ACT nor DVE is the
                    # bottleneck.
                    if a % 2 == 0:
                        nc.scalar.copy(ot[:, i, :], po[:, :])
                    else:
                        nc.vector.tensor_copy(ot[:, i, :], po[:, :])
                nc.sync.dma_start(
                    orr[:, s * OC : (s + 1) * OC, :], ot[:, :, :]
                )

    nc.finalize()
    return nc


def get_nc():
    if "nc" not in _cached:
        _cached["nc"] = _build_nc()
    return _cached["nc"]


# Tap offsets in flattened hw space (i-1)*W + (j-1), torch Unfold order.
_DELTAS = [(t // K - 1) * W + (t % K - 1) for t in range(T)]


def _pack_band(ker_n):
    """[1, T, H, W] f32 -> (bandC [P,NT*P], bandL, bandR [HR,NT*P]) bf16.

    Band B[a, b][q, p] = k_t[128a+p] where 128(a+b-1)+q == 128a+p+d_t,
    with w-edge columns of j==0/j==2 taps zeroed (kills w wraparound) and
    h-out-of-range taps dropped (zero padding).  b==0 (L) only has rows
    q in [63,128) and b==2 (R) only rows q in [0,65), so L is shipped
    shifted up by 63 and R as its first 65 rows.
    """
    k = np.array(ker_n.reshape(T, H, W), dtype=np.float32)
    for t in range(T):
        j = t % K
        if j == 0:
            k[t, :, 0] = 0.0
        elif j == K - 1:
            k[t, :, W - 1] = 0.0
    kf = k.reshape(T, HW)

    band = np.zeros((NT, 3, P, P), dtype=np.float32)  # [a, b, q, p]
    g = np.arange(HW)
    a, p = g >> 7, g & 127
    for t in range(T):
        gs = g + _DELTAS[t]
        v = (gs >= 0) & (gs < HW)
        q, b = gs & 127, (gs >> 7) - a + 1
        band[a[v], b[v], q[v], p[v]] = kf[t, v]
    def pack(blk):  # [a, q', p] -> [q', (a p)] bf16
        return np.ascontiguousarray(blk.transpose(1, 0, 2)).reshape(
            blk.shape[1], NT * P
        ).astype(ml_dtypes.bfloat16)

    return (
        pack(band[:, 1]),
        pack(band[:, 0, P - HR : P, :]),
        pack(band[:, 2, 0:HR, :]),
    )


def kernel(x, kernel, kernel_size=3, dilation=1, **_):
    x = np.asarray(x, dtype=np.float32)
    ker = np.asarray(kernel, dtype=np.float32)
    assert x.shape == (N, C, H, W), x.shape
    assert ker.shape == (N, 1, T, H, W), ker.shape

    nc = get_nc()
    in_maps = []
    for n in range(N):
        bandC, bandL, bandR = _pack_band(ker[n])
        in_maps.append(
            {
                "xT": np.ascontiguousarray(
                    x[n].reshape(C, HW).T.astype(ml_dtypes.bfloat16)
                ),
                "bandC": bandC,
                "bandL": bandL,
                "bandR": bandR,
            }
        )
    res = run_bass_kernel_spmd(
        nc,
        in_maps,
        list(range(N)),
        trace=bool(int(os.environ.get("DDC_TRACE", "0"))),
    )
    _cached["last_results"] = res
    out = np.stack(
        [
            np.asarray(res.results[n]["outT"], dtype=np.float32).T.reshape(
                C, H, W
            )
            for n in range(N)
        ]
    )
    return out

